# revision 4
# baseline (speedup 1.0000x reference)
"""Differentiable-histogram Trainium2 kernel (256 triangular bins).

Contract: kernel(**inputs) takes the FULL inputs from setup_inputs()
(images_batch: (8,3,256,256) f32 in [0,1]; bin_centers: (256,) f32 =
linspace(0,1,256), implied by the math below) and returns the FULL
(8,256) f32 histogram, matching

    hist[b, j] = sum_i relu(1 - |255*x_bi - j|)

Strategy (pure data parallel, one image per NeuronCore, 8 cores), the
"steps x ramps / second-difference" scheme:

  t = fp16(255*x)  (single rounding — casting x to fp16 first creates a
  double-quantization moire with ~10x the error), j = 16h + l.

  lhsT (coarse): S[i, a] = (t_i >= 16a) for even a, (t_i > 16a) for odd
  a, a = 0..15 (col 0 is the all-ones count row; the parity matches
  round-half-even tie behavior at exact fp16 multiples of 16).
  rhs (fine):   R[i, m] = max(r_i, m), r = t - h16, h16 = magic-rounded
  t to a multiple of 16; m = [-1, 1..15, 16] (col 0 = r, col 16 = 16 =
  16*count — the triangle is the second difference of ramps:
  tri(r-l) = max(r,l-1) - 2 max(r,l) + max(r,l+1), so the host only
  needs ramp sums; no relu/abs/clamp pass exists on the device at all).

  Device: G[a, m] = sum_i S[i,a]*R[i,m] via TensorE, 4 pair-groups per
  matmul (lhsT 128 cols = max stationary, rhs 136), PSUM-accumulated in
  4 banks. Host: difference the step rows into bucket rows, second-
  difference the ramp columns into triangle sums, fold l=16 spill.

  Engines: ScalarE computes t and ten step columns as deep-saturated
  sigmoids (exactly 0/1 in fp16); DVE computes h16/r (tensor_scalar 4x),
  six step columns (tensor_tensor 2x) and the 17-wide ramp op; x loads
  ride the GpSimd DMA queue so they don't serialize behind const loads;
  junk warm-up matmuls spin the PE out of its low p-state during the
  DMA/compute ramp.
"""

import json as _json
from contextlib import ExitStack

import numpy as np

import concourse.bass as bass
import concourse.tile as tile
from concourse import mybir
from concourse.bass_utils import run_bass_kernel_spmd

FP32 = mybir.dt.float32
FP16 = mybir.dt.float16
ALU = mybir.AluOpType
ACT = mybir.ActivationFunctionType

N_CORES = 8
P, F = 128, 1536  # per-core pixels: 3*256*256 = 196608 = 128*1536
NW = 16  # steps (coarse) width
MW = 17  # ramps (fine) width
FOLD2 = 4  # pair-groups per matmul
N_PSUM = 4  # banks per PSUM generation (two generations: 8 banks total)
N_BANKS = 8
BUFS = 3
K_DVE = 6
WARMUP_MM = 70
CHUNKS = [64, 384, 544, 544]


def _split_multiwaits(bir_bytes: bytes) -> bytes:
    """This container's walrus rejects any instruction carrying more than
    one sem wait. Split extras onto standalone EventSemaphore instructions;
    additionally drop the exit-drain's queue waits (NRT drains rings at
    exec end anyway)."""
    bir = _json.loads(bir_bytes)
    for fn in bir["functions"]:
        for blk in fn["blocks"]:
            is_end = str(blk.get("name", "")).endswith("_end")
            out = []
            for ins in blk["instructions"]:
                si = ins.get("sync_info")
                ow = (si or {}).get("on_wait") or []
                if is_end and ins.get("opcode") == "Drain" and len(ow) > 1:
                    si["on_wait"] = []
                elif len(ow) > 1:
                    for k, w in enumerate(ow[:-1]):
                        out.append(
                            {
                                "debug": ins.get("debug", 1),
                                "engine": ins["engine"],
                                "ins": [],
                                "name": f"{ins['name']}_w{k}",
                                "opcode": "EventSemaphore",
                                "outs": [],
                                "sync_info": {"on_update": [], "on_wait": [w]},
                            }
                        )
                    si["on_wait"] = [ow[-1]]
                out.append(ins)
            blk["instructions"] = out
    return _json.dumps(bir).encode()


def _src_pairs(ap, w, g2):
    # (P, Gc) tile viewed as (P, g2, w, 2): pixel pairs inner, bcast w
    return bass.AP(
        tensor=ap.tensor,
        offset=ap.offset,
        ap=[ap.ap[0], [2, g2], [0, w], [1, 2]],
    )


def _iota_bcast(ap, w, g2, w_off=0, w_stride=2):
    # (P, w_total, 2) const tile viewed as (P, g2, w, 2) from column w_off
    return bass.AP(
        tensor=ap.tensor,
        offset=ap.offset + 2 * w_off,
        ap=[ap.ap[0], [0, g2], [w_stride, w], [1, 2]],
    )


def _steps_cols(ap, a0, n, g2):
    # (P, G2, 16, 2) steps tile: columns a0, a0+2, ... (parity-strided)
    return bass.AP(
        tensor=ap.tensor,
        offset=ap.offset + 2 * a0,
        ap=[ap.ap[0], [32, g2], [4, n], [1, 2]],
    )


def _build_program():
    chunk_sizes = CHUNKS
    assert sum(chunk_sizes) == F
    for c in chunk_sizes:
        assert c % (2 * FOLD2) == 0, c

    MR, MC = NW * FOLD2 * 2, MW * FOLD2 * 2  # 128, 136
    n_mm = sum(c // (2 * FOLD2) for c in chunk_sizes)
    # generation A = all chunks but the last (banks 0-3, drained while the
    # last chunk computes); generation B = last chunk (banks 4-7)
    n_mm_a = sum(c // (2 * FOLD2) for c in chunk_sizes[:-1])
    n_mm_b = n_mm - n_mm_a

    # enable_partition_id=False: the kernel never reads the partition id,
    # and dropping it removes the per-engine preamble register loads.
    nc = bass.Bass("TRN2", target_bir_lowering=False, enable_partition_id=False)
    x_dram = nc.dram_tensor("x", [P, F], FP32, kind="ExternalInput")
    gacc_dram = nc.dram_tensor("gacc", [N_BANKS, MR, MC], FP32, kind="ExternalOutput")

    thr_np = np.array([-1.0] + [16.0 * a for a in range(1, 16)], dtype=np.float16)
    thr_np = np.broadcast_to(
        np.repeat(thr_np, 2).reshape(NW, 2)[None], (P, NW, 2)
    ).astype(np.float16)
    m_np = np.array([-1.0] + list(range(1, 16)) + [16.0], dtype=np.float16)
    m_np = np.broadcast_to(
        np.repeat(m_np, 2).reshape(MW, 2)[None], (P, MW, 2)
    ).astype(np.float16)

    with tile.TileContext(nc) as tc, ExitStack() as ctx:
        singles = ctx.enter_context(tc.tile_pool(name="singles", bufs=1))
        pool = ctx.enter_context(tc.tile_pool(name="work", bufs=BUFS))
        psum_pool = ctx.enter_context(tc.tile_pool(name="psum", bufs=1, space="PSUM"))
        out_pool = ctx.enter_context(tc.tile_pool(name="outp", bufs=1))

        thr = singles.tile([P, NW, 2], FP16)
        iom = singles.tile([P, MW, 2], FP16)
        nc.sync.dma_start(thr[:], nc.inline_tensor(np.ascontiguousarray(thr_np), "thr")[:])
        nc.sync.dma_start(iom[:], nc.inline_tensor(np.ascontiguousarray(m_np), "iom")[:])
        # ScalarE step-column biases: Sigmoid(SSC*(t - 16a +- d)) is exactly
        # 0/1 in fp16 for every representable t; +-d encodes is_ge/is_gt at
        # exact fp16 ties, matching the DVE columns' parity behavior.
        SSC = 4096.0
        bias_np = np.array(
            [SSC * (-16.0 * a + (0.004 if a % 2 == 0 else -0.004)) for a in range(NW)],
            dtype=np.float32,
        )
        bias_np = np.broadcast_to(bias_np[None], (P, NW)).astype(np.float32)
        sbias = singles.tile([P, NW], FP32)
        nc.sync.dma_start(sbias[:], nc.inline_tensor(np.ascontiguousarray(bias_np), "sbias")[:])

        psums = [
            psum_pool.tile([MR, MC], FP32, tag=f"ps{i}", name=f"ps{i}")
            for i in range(N_BANKS)
        ]

        if WARMUP_MM:
            # Spin the PE out of its low p-state while DMA/DVE ramp up. All 8
            # banks are in use: borrow the last gen-B bank (its real
            # accumulation group later opens with start=True, resetting it).
            wps_ap = psums[N_BANKS - 1][0 : NW * 2, :]
            for i in range(WARMUP_MM):
                nc.tensor.matmul(
                    wps_ap,
                    thr[:],
                    _iota_bcast(iom[:], MW, FOLD2),
                    start=(i == 0),
                    stop=(i == WARMUP_MM - 1),
                    skip_group_check=True,
                )

        mi = 0
        x_off = 0
        for ci, Gc in enumerate(chunk_sizes):
            G2 = Gc // 2
            xc = pool.tile([P, Gc], FP32, tag="xc")
            # x loads ride the otherwise-idle GpSimd DMA queue so their issue
            # doesn't serialize behind the const loads on the sync queue.
            nc.gpsimd.dma_start(xc[:], x_dram[:, x_off : x_off + Gc])
            x_off += Gc

            t = pool.tile([P, Gc], FP16, tag="t")
            nc.scalar.activation(t[:], xc[:], ACT.Copy, scale=255.0)
            # h16 = round-to-multiple-of-16(t - 8) via f32 magic; exact fp16
            # functions of t, so routing is self-consistent. r = t - h16.
            M16 = 12582912.0 * 16.0
            w = pool.tile([P, Gc], FP16, tag="w")
            nc.vector.tensor_scalar(w[:], t[:], 8.0, 0.0, ALU.subtract, ALU.add)
            h = pool.tile([P, Gc], FP16, tag="h")
            nc.vector.tensor_scalar(h[:], w[:], M16, M16, ALU.add, ALU.subtract)
            r = pool.tile([P, Gc], FP16, tag="r")
            nc.vector.tensor_tensor(r[:], t[:], h[:], ALU.subtract)

            # Steps: col a = (t >= 16a) for even a, (t > 16a) for odd a.
            # Small chunks keep all columns on DVE (ScalarE's ~450ns
            # per-instruction overhead dwarfs the tiny bodies there).
            kd = NW if Gc <= 256 else K_DVE
            steps = pool.tile([P, G2, NW, 2], FP16, tag="steps")
            ke, ko = (kd + 1) // 2, kd // 2
            nc.vector.tensor_tensor(
                _steps_cols(steps[:], 0, ke, G2),
                _src_pairs(t[:], ke, G2),
                _iota_bcast(thr[:], ke, G2, w_off=0, w_stride=4),
                ALU.is_ge,
            )
            nc.vector.tensor_tensor(
                _steps_cols(steps[:], 1, ko, G2),
                _src_pairs(t[:], ko, G2),
                _iota_bcast(thr[:], ko, G2, w_off=1, w_stride=4),
                ALU.is_gt,
            )
            for a in range(kd, NW):
                col_out = bass.AP(
                    tensor=steps.tensor,
                    offset=steps[:].offset + 2 * a,
                    ap=[steps[:].ap[0], [2 * NW, G2], [1, 2]],
                )
                col_in = bass.AP(
                    tensor=t.tensor,
                    offset=t[:].offset,
                    ap=[t[:].ap[0], [2, G2], [1, 2]],
                )
                nc.scalar.activation(
                    col_out, col_in, ACT.Sigmoid, bias=sbias[:, a : a + 1],
                    scale=SSC,
                )

            ramps = pool.tile([P, G2, MW, 2], FP16, tag="ramps")
            nc.vector.tensor_tensor(
                ramps[:],
                _src_pairs(r[:], MW, G2),
                _iota_bcast(iom[:], MW, G2),
                ALU.max,
            )

            if ci == len(chunk_sizes) - 1:
                for gb in range(0, G2, FOLD2):
                    mb = mi - n_mm_a
                    nc.tensor.matmul(
                        psums[N_PSUM + mb % N_PSUM][:],
                        steps[:, gb : gb + FOLD2, :, :],
                        ramps[:, gb : gb + FOLD2, :, :],
                        start=(mb < N_PSUM),
                        stop=(mb >= n_mm_b - N_PSUM),
                    )
                    mi += 1
            else:
                for gb in range(0, G2, FOLD2):
                    nc.tensor.matmul(
                        psums[mi % N_PSUM][:],
                        steps[:, gb : gb + FOLD2, :, :],
                        ramps[:, gb : gb + FOLD2, :, :],
                        start=(mi < N_PSUM),
                        stop=(mi >= n_mm_a - N_PSUM),
                    )
                    mi += 1
            if ci == len(chunk_sizes) - 2:
                # drain generation A while the last chunk computes
                stage = out_pool.tile([MR, N_BANKS, MC], FP32)
                for i in range(N_PSUM):
                    nc.scalar.activation(stage[:, i, :], psums[i][:], ACT.Copy)
        assert mi == n_mm

        for i in range(N_PSUM, N_BANKS):
            if i % 2 == 0:
                nc.vector.tensor_copy(stage[:, i, :], psums[i][:])
            else:
                nc.scalar.activation(stage[:, i, :], psums[i][:], ACT.Copy)
        nc.sync.dma_start(gacc_dram.rearrange("n r c -> r n c"), stage[:])

    orig = nc.to_json_bytes
    nc.to_json_bytes = lambda *a, **k: _split_multiwaits(orig(*a, **k))
    return nc


def _gacc_to_hist(gacc: np.ndarray) -> np.ndarray:
    """(N_BANKS, 128, 136) raw PSUM accumulators -> (256,) histogram."""
    acc = gacc.astype(np.float64).sum(axis=0)  # (128, 136)
    G = np.zeros((NW, MW), np.float64)
    for g in range(FOLD2):
        for p in range(2):
            G += acc[g * 32 + p : g * 32 + 32 : 2, g * 34 + p : g * 34 + 34 : 2]
    # step rows -> bucket rows
    Gd = G - np.vstack([G[1:], np.zeros((1, MW))])

    def Rm(h, m):
        if m <= 0:
            return Gd[h, 0]
        if m <= 15:
            return Gd[h, m]
        if m == 16:
            return Gd[h, 16]
        return Gd[h, 16] * 17.0 / 16.0

    hist = np.zeros(257)
    for h in range(16):
        for l in range(17):
            hist[16 * h + l] += Rm(h, l - 1) - 2 * Rm(h, l) + Rm(h, l + 1)
    return hist[:256].astype(np.float32)


_NC_CACHE = []


def kernel(images_batch: np.ndarray, bin_centers: np.ndarray) -> np.ndarray:
    images = np.asarray(images_batch, dtype=np.float32)
    assert images.shape == (N_CORES, 3, 256, 256), images.shape
    # bin_centers is linspace(0,1,256) by construction; the kernel math
    # hardcodes those bins (t = 255*x vs integer bin index).

    if not _NC_CACHE:
        _NC_CACHE.append(_build_program())
    nc = _NC_CACHE[0]

    in_maps = [{"x": images[b].reshape(P, F).copy()} for b in range(N_CORES)]
    res = run_bass_kernel_spmd(nc, in_maps, core_ids=list(range(N_CORES)))
    return np.stack([_gacc_to_hist(res.results[b]["gacc"]) for b in range(N_CORES)])


if __name__ == "__main__":
    rng = np.random.default_rng(1)
    imgs = rng.random((8, 3, 256, 256), dtype=np.float32)
    bins = np.linspace(0.0, 1.0, 256, dtype=np.float32)
    out = kernel(images_batch=imgs, bin_centers=bins)
    t = imgs.reshape(8, -1).astype(np.float64) * 255.0
    j = np.arange(256)
    want = np.clip(1.0 - np.abs(t[:, :, None] - j[None, None, :]), 0, None).sum(1)
    rel = np.abs(out - want).max() / np.abs(want).max()
    print("self-test rel err:", rel)
    print("PASS" if rel < 2e-2 else "FAIL")


# revision 5
# speedup vs baseline: 1.0379x; 1.0379x over previous
"""Differentiable-histogram Trainium2 kernel (256 triangular bins).

Contract: kernel(**inputs) takes the FULL inputs from setup_inputs()
(images_batch: (8,3,256,256) f32 in [0,1]; bin_centers: (256,) f32 =
linspace(0,1,256), implied by the math below) and returns the FULL
(8,256) f32 histogram, matching

    hist[b, j] = sum_i relu(1 - |255*x_bi - j|)

Strategy (pure data parallel, one image per NeuronCore, 8 cores), the
"steps x ramps / second-difference" scheme:

  t = fp16(255*x)  (single rounding — casting x to fp16 first creates a
  double-quantization moire with ~10x the error), j = 16h + l.

  lhsT (coarse): S[i, a] = (t_i >= 16a) for even a, (t_i > 16a) for odd
  a, a = 0..15 (col 0 is the all-ones count row; the parity matches
  round-half-even tie behavior at exact fp16 multiples of 16).
  rhs (fine):   R[i, m] = max(r_i, m), r = t - h16, h16 = magic-rounded
  t to a multiple of 16; m = [-1, 1..15, 16] (col 0 = r, col 16 = 16 =
  16*count — the triangle is the second difference of ramps:
  tri(r-l) = max(r,l-1) - 2 max(r,l) + max(r,l+1), so the host only
  needs ramp sums; no relu/abs/clamp pass exists on the device at all).

  Device: G[a, m] = sum_i S[i,a]*R[i,m] via TensorE, 4 pair-groups per
  matmul (lhsT 128 cols = max stationary, rhs 136), PSUM-accumulated in
  4 banks. Host: difference the step rows into bucket rows, second-
  difference the ramp columns into triangle sums, fold l=16 spill.

  Engines: ScalarE computes t and ten step columns as deep-saturated
  sigmoids (exactly 0/1 in fp16); DVE computes h16/r (tensor_scalar 4x),
  six step columns (tensor_tensor 2x) and the 17-wide ramp op; x loads
  ride the GpSimd DMA queue so they don't serialize behind const loads;
  junk warm-up matmuls spin the PE out of its low p-state during the
  DMA/compute ramp.
"""

import json as _json
from contextlib import ExitStack

import numpy as np

import concourse.bass as bass
import concourse.tile as tile
from concourse import mybir
from concourse.bass_utils import run_bass_kernel_spmd

FP32 = mybir.dt.float32
FP16 = mybir.dt.float16
ALU = mybir.AluOpType
ACT = mybir.ActivationFunctionType

N_CORES = 8
P, F = 128, 1536  # per-core pixels: 3*256*256 = 196608 = 128*1536
NW = 16  # steps (coarse) width
MW = 17  # ramps (fine) width
FOLD2 = 4  # pair-groups per matmul
N_PSUM = 4  # banks per PSUM generation (two generations: 8 banks total)
N_BANKS = 8
BUFS = 3
K_DVE = 6
WARMUP_MM = 70
CHUNKS = [64, 384, 544, 544]


def _split_multiwaits(bir_bytes: bytes) -> bytes:
    """This container's walrus rejects any instruction carrying more than
    one sem wait. Split extras onto standalone EventSemaphore instructions;
    additionally drop the exit-drain's queue waits (NRT drains rings at
    exec end anyway)."""
    bir = _json.loads(bir_bytes)
    for fn in bir["functions"]:
        for blk in fn["blocks"]:
            is_end = str(blk.get("name", "")).endswith("_end")
            out = []
            for ins in blk["instructions"]:
                si = ins.get("sync_info")
                ow = (si or {}).get("on_wait") or []
                if is_end and ins.get("opcode") == "Drain" and len(ow) > 1:
                    si["on_wait"] = []
                elif len(ow) > 1:
                    for k, w in enumerate(ow[:-1]):
                        out.append(
                            {
                                "debug": ins.get("debug", 1),
                                "engine": ins["engine"],
                                "ins": [],
                                "name": f"{ins['name']}_w{k}",
                                "opcode": "EventSemaphore",
                                "outs": [],
                                "sync_info": {"on_update": [], "on_wait": [w]},
                            }
                        )
                    si["on_wait"] = [ow[-1]]
                out.append(ins)
            blk["instructions"] = out
    return _json.dumps(bir).encode()


def _src_pairs(ap, w, g2):
    # (P, Gc) tile viewed as (P, g2, w, 2): pixel pairs inner, bcast w
    return bass.AP(
        tensor=ap.tensor,
        offset=ap.offset,
        ap=[ap.ap[0], [2, g2], [0, w], [1, 2]],
    )


def _iota_bcast(ap, w, g2, w_off=0, w_stride=2):
    # (P, w_total, 2) const tile viewed as (P, g2, w, 2) from column w_off
    return bass.AP(
        tensor=ap.tensor,
        offset=ap.offset + 2 * w_off,
        ap=[ap.ap[0], [0, g2], [w_stride, w], [1, 2]],
    )


def _steps_cols(ap, a0, n, g2):
    # (P, G2, 16, 2) steps tile: columns a0, a0+2, ... (parity-strided)
    return bass.AP(
        tensor=ap.tensor,
        offset=ap.offset + 2 * a0,
        ap=[ap.ap[0], [32, g2], [4, n], [1, 2]],
    )


def _build_program():
    chunk_sizes = CHUNKS
    assert sum(chunk_sizes) == F
    for c in chunk_sizes:
        assert c % (2 * FOLD2) == 0, c

    MR, MC = NW * FOLD2 * 2, MW * FOLD2 * 2  # 128, 136
    n_mm = sum(c // (2 * FOLD2) for c in chunk_sizes)
    # generation A = all chunks but the last (banks 0-3, drained while the
    # last chunk computes); generation B = last chunk (banks 4-7)
    n_mm_a = sum(c // (2 * FOLD2) for c in chunk_sizes[:-1])
    n_mm_b = n_mm - n_mm_a

    # enable_partition_id=False: the kernel never reads the partition id,
    # and dropping it removes the per-engine preamble register loads.
    nc = bass.Bass("TRN2", target_bir_lowering=False, enable_partition_id=False)
    x_dram = nc.dram_tensor("x", [P, F], FP32, kind="ExternalInput")
    gacc_dram = nc.dram_tensor("gacc", [N_BANKS, MR, MC], FP32, kind="ExternalOutput")

    thr_np = np.array([-1.0] + [16.0 * a for a in range(1, 16)], dtype=np.float16)
    thr_np = np.broadcast_to(
        np.repeat(thr_np, 2).reshape(NW, 2)[None], (P, NW, 2)
    ).astype(np.float16)
    m_np = np.array([-1.0] + list(range(1, 16)) + [16.0], dtype=np.float16)
    m_np = np.broadcast_to(
        np.repeat(m_np, 2).reshape(MW, 2)[None], (P, MW, 2)
    ).astype(np.float16)

    with tile.TileContext(nc) as tc, ExitStack() as ctx:
        singles = ctx.enter_context(tc.tile_pool(name="singles", bufs=1))
        pool = ctx.enter_context(tc.tile_pool(name="work", bufs=BUFS))
        psum_pool = ctx.enter_context(tc.tile_pool(name="psum", bufs=1, space="PSUM"))
        out_pool = ctx.enter_context(tc.tile_pool(name="outp", bufs=1))

        thr = singles.tile([P, NW, 2], FP16)
        iom = singles.tile([P, MW, 2], FP16)
        nc.sync.dma_start(thr[:], nc.inline_tensor(np.ascontiguousarray(thr_np), "thr")[:])
        nc.sync.dma_start(iom[:], nc.inline_tensor(np.ascontiguousarray(m_np), "iom")[:])
        # ScalarE step-column biases: Sigmoid(SSC*(t - 16a +- d)) is exactly
        # 0/1 in fp16 for every representable t; +-d encodes is_ge/is_gt at
        # exact fp16 ties, matching the DVE columns' parity behavior.
        SSC = 4096.0
        bias_np = np.array(
            [SSC * (-16.0 * a + (0.004 if a % 2 == 0 else -0.004)) for a in range(NW)],
            dtype=np.float32,
        )
        bias_np = np.broadcast_to(bias_np[None], (P, NW)).astype(np.float32)
        sbias = singles.tile([P, NW], FP32)
        nc.sync.dma_start(sbias[:], nc.inline_tensor(np.ascontiguousarray(bias_np), "sbias")[:])

        psums = [
            psum_pool.tile([MR, MC], FP32, tag=f"ps{i}", name=f"ps{i}")
            for i in range(N_BANKS)
        ]

        if WARMUP_MM:
            # Spin the PE out of its low p-state while DMA/DVE ramp up. All 8
            # banks are in use: borrow the last gen-B bank (its real
            # accumulation group later opens with start=True, resetting it).
            wps_ap = psums[N_BANKS - 1][0 : NW * 2, :]
            for i in range(WARMUP_MM):
                nc.tensor.matmul(
                    wps_ap,
                    thr[:],
                    _iota_bcast(iom[:], MW, FOLD2),
                    start=(i == 0),
                    stop=(i == WARMUP_MM - 1),
                    skip_group_check=True,
                )

        mi = 0
        x_off = 0
        for ci, Gc in enumerate(chunk_sizes):
            G2 = Gc // 2
            xc = pool.tile([P, Gc], FP32, tag="xc")
            # x loads ride the otherwise-idle GpSimd DMA queue so their issue
            # doesn't serialize behind the const loads on the sync queue.
            nc.gpsimd.dma_start(xc[:], x_dram[:, x_off : x_off + Gc])
            x_off += Gc

            t = pool.tile([P, Gc], FP16, tag="t")
            if ci == 0:
                # first chunk: DVE computes t so the ScalarE ACT-table load
                # and first Copy stay off the startup critical path
                nc.vector.tensor_scalar(t[:], xc[:], 255.0, 0.0, ALU.mult, ALU.add)
            else:
                nc.scalar.activation(t[:], xc[:], ACT.Copy, scale=255.0)
            # h16 = round-to-multiple-of-16(t - 8) via f32 magic; exact fp16
            # functions of t, so routing is self-consistent. r = t - h16.
            M16 = 12582912.0 * 16.0
            w = pool.tile([P, Gc], FP16, tag="w")
            nc.vector.tensor_scalar(w[:], t[:], 8.0, 0.0, ALU.subtract, ALU.add)
            h = pool.tile([P, Gc], FP16, tag="h")
            nc.vector.tensor_scalar(h[:], w[:], M16, M16, ALU.add, ALU.subtract)
            r = pool.tile([P, Gc], FP16, tag="r")
            nc.vector.tensor_tensor(r[:], t[:], h[:], ALU.subtract)

            # Steps: col a = (t >= 16a) for even a, (t > 16a) for odd a.
            # Small chunks keep all columns on DVE (ScalarE's ~450ns
            # per-instruction overhead dwarfs the tiny bodies there).
            kd = NW if Gc <= 256 else K_DVE
            steps = pool.tile([P, G2, NW, 2], FP16, tag="steps")
            ke, ko = (kd + 1) // 2, kd // 2
            nc.vector.tensor_tensor(
                _steps_cols(steps[:], 0, ke, G2),
                _src_pairs(t[:], ke, G2),
                _iota_bcast(thr[:], ke, G2, w_off=0, w_stride=4),
                ALU.is_ge,
            )
            nc.vector.tensor_tensor(
                _steps_cols(steps[:], 1, ko, G2),
                _src_pairs(t[:], ko, G2),
                _iota_bcast(thr[:], ko, G2, w_off=1, w_stride=4),
                ALU.is_gt,
            )
            for a in range(kd, NW):
                col_out = bass.AP(
                    tensor=steps.tensor,
                    offset=steps[:].offset + 2 * a,
                    ap=[steps[:].ap[0], [2 * NW, G2], [1, 2]],
                )
                col_in = bass.AP(
                    tensor=t.tensor,
                    offset=t[:].offset,
                    ap=[t[:].ap[0], [2, G2], [1, 2]],
                )
                nc.scalar.activation(
                    col_out, col_in, ACT.Sigmoid, bias=sbias[:, a : a + 1],
                    scale=SSC,
                )

            ramps = pool.tile([P, G2, MW, 2], FP16, tag="ramps")
            nc.vector.tensor_tensor(
                ramps[:],
                _src_pairs(r[:], MW, G2),
                _iota_bcast(iom[:], MW, G2),
                ALU.max,
            )

            if ci == len(chunk_sizes) - 1:
                # drain generation A now: these copies run on DVE/SE while
                # the PE crunches the last chunk's matmuls
                stage = out_pool.tile([MR, N_BANKS, MC], FP32)
                nc.vector.tensor_copy(stage[:, 0, :], psums[0][:])
                nc.vector.tensor_copy(stage[:, 1, :], psums[1][:])
                nc.scalar.activation(stage[:, 2, :], psums[2][:], ACT.Copy)
                nc.scalar.activation(stage[:, 3, :], psums[3][:], ACT.Copy)
                for gb in range(0, G2, FOLD2):
                    mb = mi - n_mm_a
                    nc.tensor.matmul(
                        psums[N_PSUM + mb % N_PSUM][:],
                        steps[:, gb : gb + FOLD2, :, :],
                        ramps[:, gb : gb + FOLD2, :, :],
                        start=(mb < N_PSUM),
                        stop=(mb >= n_mm_b - N_PSUM),
                    )
                    mi += 1
            else:
                for gb in range(0, G2, FOLD2):
                    nc.tensor.matmul(
                        psums[mi % N_PSUM][:],
                        steps[:, gb : gb + FOLD2, :, :],
                        ramps[:, gb : gb + FOLD2, :, :],
                        start=(mi < N_PSUM),
                        stop=(mi >= n_mm_a - N_PSUM),
                    )
                    mi += 1
        assert mi == n_mm

        for i in range(N_PSUM, N_BANKS):
            if i % 2 == 0:
                nc.vector.tensor_copy(stage[:, i, :], psums[i][:])
            else:
                nc.scalar.activation(stage[:, i, :], psums[i][:], ACT.Copy)
        nc.sync.dma_start(gacc_dram.rearrange("n r c -> r n c"), stage[:])

    orig = nc.to_json_bytes
    nc.to_json_bytes = lambda *a, **k: _split_multiwaits(orig(*a, **k))
    return nc


def _gacc_to_hist(gacc: np.ndarray) -> np.ndarray:
    """(N_BANKS, 128, 136) raw PSUM accumulators -> (256,) histogram."""
    acc = gacc.astype(np.float64).sum(axis=0)  # (128, 136)
    G = np.zeros((NW, MW), np.float64)
    for g in range(FOLD2):
        for p in range(2):
            G += acc[g * 32 + p : g * 32 + 32 : 2, g * 34 + p : g * 34 + 34 : 2]
    # step rows -> bucket rows
    Gd = G - np.vstack([G[1:], np.zeros((1, MW))])

    def Rm(h, m):
        if m <= 0:
            return Gd[h, 0]
        if m <= 15:
            return Gd[h, m]
        if m == 16:
            return Gd[h, 16]
        return Gd[h, 16] * 17.0 / 16.0

    hist = np.zeros(257)
    for h in range(16):
        for l in range(17):
            hist[16 * h + l] += Rm(h, l - 1) - 2 * Rm(h, l) + Rm(h, l + 1)
    return hist[:256].astype(np.float32)


_NC_CACHE = []


def kernel(images_batch: np.ndarray, bin_centers: np.ndarray) -> np.ndarray:
    images = np.asarray(images_batch, dtype=np.float32)
    assert images.shape == (N_CORES, 3, 256, 256), images.shape
    # bin_centers is linspace(0,1,256) by construction; the kernel math
    # hardcodes those bins (t = 255*x vs integer bin index).

    if not _NC_CACHE:
        _NC_CACHE.append(_build_program())
    nc = _NC_CACHE[0]

    in_maps = [{"x": images[b].reshape(P, F).copy()} for b in range(N_CORES)]
    res = run_bass_kernel_spmd(nc, in_maps, core_ids=list(range(N_CORES)))
    return np.stack([_gacc_to_hist(res.results[b]["gacc"]) for b in range(N_CORES)])


if __name__ == "__main__":
    rng = np.random.default_rng(1)
    imgs = rng.random((8, 3, 256, 256), dtype=np.float32)
    bins = np.linspace(0.0, 1.0, 256, dtype=np.float32)
    out = kernel(images_batch=imgs, bin_centers=bins)
    t = imgs.reshape(8, -1).astype(np.float64) * 255.0
    j = np.arange(256)
    want = np.clip(1.0 - np.abs(t[:, :, None] - j[None, None, :]), 0, None).sum(1)
    rel = np.abs(out - want).max() / np.abs(want).max()
    print("self-test rel err:", rel)
    print("PASS" if rel < 2e-2 else "FAIL")
